# revision 20
# baseline (speedup 1.0000x reference)
"""Trainium2 Bass kernel for nn_MemoryAccess (scatter_memory).

Uniform SPMD program on 8 cores (no control flow); per-core behavior is
driven purely by per-core input data:
  - every core holds a 1250-column slice (x3 s-blocks) of the read
    projection (fp32) and computes logits + softmax stats (max / sumexp /
    argmax) for its slice.
  - every core mirrors the update-head + chain code on its own weight
    slots; the slots hold real weights only on their owner core
    (core 5: upd_mem + upd_w, core 6: upd_mem_rand + upd_w_rand,
    core 7: apply_mem + apply_mem_rand), zeros elsewhere.
  - gather indices select between the computed argmax (core 5, mask=1)
    and the host-prepacked random indices (all other cores, mask=0).
  - 4 small AllGathers: softmax stats, q partials (2nd softmax), r/r2
    vectors for s=0, then s=1,2. Core 7 runs the serial 6-step m chain.
LayerNorm in every head is folded into the following matmul: two extra
k-rows (mu x -colsum(W'), sqrt(var+eps) x c0-row) plus a per-row rstd
scale fused into the Relu/Sigmoid activation (relu(s*x)=s*relu(x), s>0).
"""

import os

os.environ.setdefault("JAX_PLATFORMS", "cpu")

import numpy as np
import ml_dtypes

IN_CH = 512
SLOT = 512
FEAT = 16
AVAIL = 10000
RS = 3
B = 64
NC = 8

CW = AVAIL // NC        # 1250 cols per (core, s-block)
U_CORE = 5              # upd_mem + upd_w owner (uses computed argmax gather)
U2_CORE = 6             # upd_mem_rand + upd_w_rand owner (random gather)
CH_CORE = 7             # apply_mem + apply_mem_rand owner; runs the chain

F32 = np.float32
BF16 = ml_dtypes.bfloat16
F1 = 12   # AG1 cols: m_chunk[3] z_chunk[3] gidx[3] pad[3]
F2 = 4    # AG2 cols: q[3] pad


def _pack_head(p, dtype):
    fa = p["fa"]
    W1 = np.concatenate([np.asarray(fa["W1"], F32),
                         np.asarray(fa["b1"], F32)[None, :]], 0)
    W2 = np.concatenate([np.asarray(fa["W2"], F32),
                         np.asarray(fa["b2"], F32)[None, :]], 0)
    Wf = np.concatenate([np.asarray(fa["Wf"], F32),
                         np.asarray(fa["bf"], F32)[None, :]], 0)
    Wp = np.asarray(fa["ln_g"], F32)[:, None] * np.asarray(p["W"], F32)
    c0 = (np.asarray(fa["ln_b"], F32) @ np.asarray(p["W"], F32)
          + np.asarray(p["b"], F32))
    Wn = np.concatenate([Wp, -Wp.sum(0)[None, :], c0[None, :]], 0)
    return [np.ascontiguousarray(a.astype(dtype)) for a in (W1, W2, Wf, Wn)]


def _idx16(idx):
    """(192,) row indices -> (16,12) gather layout (flat i -> [i%16, i//16])."""
    a = np.zeros((16, 12), F32)
    for i in range(192):
        a[i % 16, i // 16] = float(idx[i])
    return a


def _build_inputs(inputs, memory, random_indices, params):
    x = np.ascontiguousarray(np.asarray(inputs, F32))
    mem = np.ascontiguousarray(np.asarray(memory, F32))
    ridx = np.asarray(random_indices)
    ridx16 = _idx16(ridx.T.reshape(-1))         # order s*64+b

    rH = _pack_head(params["read_w"], F32)
    uwH = _pack_head(params["upd_w"], BF16)
    uwrH = _pack_head(params["upd_w_rand"], BF16)
    uH = _pack_head(params["upd_mem"], BF16)
    u2H = _pack_head(params["upd_mem_rand"], BF16)
    apH = _pack_head(params["apply_mem"], BF16)
    aprH = _pack_head(params["apply_mem_rand"], BF16)
    zH_big = [np.zeros_like(a) for a in uH]
    zH_sm = [np.zeros_like(a) for a in uwH]

    in_maps = []
    for c in range(NC):
        d = {"x": x, "rW1": rH[0], "rW2": rH[1], "rWf": rH[2]}
        sl = np.empty((IN_CH + 2, RS * CW), F32)
        for s in range(RS):
            sl[:, s * CW:(s + 1) * CW] = \
                rH[3][:, s * AVAIL + c * CW:s * AVAIL + (c + 1) * CW]
        d["rWslice"] = np.ascontiguousarray(sl)
        bigA = {U_CORE: uH, U2_CORE: u2H, CH_CORE: apH}.get(c, zH_big)
        bigB = aprH if c == CH_CORE else zH_big
        sm = {U_CORE: uwH, U2_CORE: uwrH}.get(c, zH_sm)
        for n, a in zip(("bW1a", "bW2a", "bWfa", "bWna"), bigA):
            d[n] = a
        for n, a in zip(("bW1b", "bW2b", "bWfb", "bWnb"), bigB):
            d[n] = a
        for n, a in zip(("sW1", "sW2", "sWf", "sWn"), sm):
            d[n] = a
        d["memory"] = mem if c in (U_CORE, U2_CORE) else \
            np.zeros((AVAIL, SLOT), F32)
        d["ridx16"] = ridx16
        d["consts"] = np.array([[float(c * CW),
                                 1.0 if c == U_CORE else 0.0, 0.0, 0.0]], F32)
        in_maps.append(d)
    return in_maps


def _build_nc():
    import concourse.bass as bass
    import concourse.mybir as mybir
    import concourse.tile as tile
    from concourse import bacc
    from concourse.masks import make_identity

    f32 = mybir.dt.float32
    bf16 = mybir.dt.bfloat16
    i16 = mybir.dt.int16
    u32 = mybir.dt.uint32
    AF = mybir.ActivationFunctionType
    ALU = mybir.AluOpType
    AX = mybir.AxisListType

    nc = bacc.Bacc(None, target_bir_lowering=False)

    def din(name, shape, dt=f32):
        return nc.dram_tensor(name, shape, dt, kind="ExternalInput")

    x_d = din("x", [B, IN_CH])
    rW1_d = din("rW1", [IN_CH + 1, FEAT])
    rW2_d = din("rW2", [FEAT + 1, IN_CH])
    rWf_d = din("rWf", [2 * IN_CH + 1, IN_CH])
    rWs_d = din("rWslice", [IN_CH + 2, RS * CW])
    bigd = {}
    for sl in "ab":
        bigd["W1" + sl] = din("bW1" + sl, [2 * SLOT + 1, FEAT], bf16)
        bigd["W2" + sl] = din("bW2" + sl, [FEAT + 1, 2 * SLOT], bf16)
        bigd["Wf" + sl] = din("bWf" + sl, [4 * SLOT + 1, 2 * SLOT], bf16)
        bigd["Wn" + sl] = din("bWn" + sl, [2 * SLOT + 2, SLOT], bf16)
    sW1_d = din("sW1", [IN_CH + 1, FEAT], bf16)
    sW2_d = din("sW2", [FEAT + 1, IN_CH], bf16)
    sWf_d = din("sWf", [2 * IN_CH + 1, IN_CH], bf16)
    sWn_d = din("sWn", [IN_CH + 2, RS], bf16)
    memory_d = din("memory", [AVAIL, SLOT])
    ridx16_d = din("ridx16", [16, 12])
    consts_d = din("consts", [1, 4])
    out_d = nc.dram_tensor("out", [B, SLOT], f32, kind="ExternalOutput")

    dbg_d = nc.dram_tensor("dbg", [B, 16], f32, kind="ExternalOutput")

    def dap(handle_or_tile, offset, ap):
        base = handle_or_tile[:]
        return bass.AP(tensor=base.tensor, offset=base.offset + offset, ap=ap)

    import contextlib

    with tile.TileContext(nc) as tc, contextlib.ExitStack() as ctx:
        wp = ctx.enter_context(tc.tile_pool(name="wp", bufs=1))
        sp = ctx.enter_context(tc.tile_pool(name="sp", bufs=2))
        work = ctx.enter_context(tc.tile_pool(name="work", bufs=2))
        small = ctx.enter_context(tc.tile_pool(name="small", bufs=2))
        ps_mm = ctx.enter_context(tc.tile_pool(name="ps_mm", bufs=1, space="PSUM"))
        ps_no = ctx.enter_context(tc.tile_pool(name="ps_no", bufs=2, space="PSUM"))
        ps_g = ctx.enter_context(tc.tile_pool(name="ps_g", bufs=1, space="PSUM"))
        ps_tp = ctx.enter_context(tc.tile_pool(name="ps_tp", bufs=2, space="PSUM"))
        dram = ctx.enter_context(tc.tile_pool(name="dram", bufs=1, space="DRAM"))
        work1 = ctx.enter_context(tc.tile_pool(name="work1", bufs=1))

        ident = wp.tile([128, 128], f32, tag="ident")
        make_identity(nc, ident[:])
        identb = wp.tile([128, 128], bf16, tag="identb")
        nc.vector.tensor_copy(identb[:], ident[:])
        ones = wp.tile([1, 2 * B], f32, tag="ones")
        nc.vector.memset(ones[:], 1.0)
        onesb = wp.tile([1, 2 * B], bf16, tag="onesb")
        nc.vector.memset(onesb[:], 1.0)
        eps_sb = wp.tile([128, 1], f32, tag="eps")
        nc.vector.memset(eps_sb[:], 1e-5)

        def transpose_in(dst, src_ap, dt, koff=0, coff=0):
            M = src_ap.shape[0]
            nk = src_ap.shape[-1] // 128
            src2 = src_ap.rearrange("m (k p) -> m k p", p=128)
            idm = ident if dt == f32 else identb
            bp = src_ap.base_partition()
            for k in range(nk):
                pt = ps_tp.tile([128, 128], dt, tag="tp")
                nc.tensor.transpose(pt[:, :M], src2[:, k, :],
                                    idm[bp:bp + M, bp:bp + M])
                nc.vector.tensor_copy(dst[:, koff + k, coff:coff + M], pt[:, :M])

        def load_head(tag, dt, W1d, W2d, Wfd, Wnd, d_in, dout):
            nk = d_in // 128
            w = {}
            w["W1"] = wp.tile([128, nk, FEAT], dt, tag=tag + "W1",
                              name=tag + "W1")
            nc.sync.dma_start(
                out=w["W1"][:],
                in_=dap(W1d, 0, [[FEAT, 128], [128 * FEAT, nk], [1, FEAT]]))
            w["W1r"] = wp.tile([1, FEAT], dt, tag=tag + "W1r",
                               name=tag + "W1r")
            nc.sync.dma_start(out=w["W1r"][:], in_=W1d[d_in:d_in + 1, :])
            w["W2"] = wp.tile([FEAT + 1, d_in], dt, tag=tag + "W2",
                              name=tag + "W2")
            nc.sync.dma_start(out=w["W2"][:], in_=W2d[:])
            wftag = "bigAWf" if tag in ("rfa", "bigA") else tag + "Wf"
            w["Wf"] = wp.tile([128, 2 * nk, d_in], dt, tag=wftag,
                              name=tag + "Wf")
            nc.sync.dma_start(
                out=w["Wf"][:],
                in_=dap(Wfd, 0, [[d_in, 128], [128 * d_in, 2 * nk], [1, d_in]]))
            w["Wfr"] = wp.tile([1, d_in], dt, tag=tag + "Wfr",
                               name=tag + "Wfr")
            nc.sync.dma_start(out=w["Wfr"][:], in_=Wfd[2 * d_in:2 * d_in + 1, :])
            if Wnd is not None:
                w["Wn"] = wp.tile([128, nk, dout], dt, tag=tag + "Wn",
                                  name=tag + "Wn")
                nc.sync.dma_start(
                    out=w["Wn"][:],
                    in_=dap(Wnd, 0, [[dout, 128], [128 * dout, nk], [1, dout]]))
                w["Wnr"] = wp.tile([2, dout], dt, tag=tag + "Wnr",
                                   name=tag + "Wnr")
                nc.sync.dma_start(out=w["Wnr"][:], in_=Wnd[d_in:d_in + 2, :])
            w["d_in"], w["dout"], w["dt"] = d_in, dout, dt
            return w

        def head_fa(w, INseg, M):
            dt = w["dt"]
            d_in = w["d_in"]
            nk_in = d_in // 128
            one_r = ones if dt == f32 else onesb
            idm = ident if dt == f32 else identb

            c_ps = ps_no.tile([M, 512], f32, tag="nout", name="c_ps")
            ki = 0
            for seg, segnk in INseg:
                for k in range(segnk):
                    nc.tensor.matmul(c_ps[:, :FEAT], seg[:, k, :M],
                                     w["W1"][:, ki, :], start=(ki == 0),
                                     stop=False)
                    ki += 1
            nc.tensor.matmul(c_ps[:, :FEAT], one_r[:, :M], w["W1r"][:],
                             start=False, stop=True)
            mx = small.tile([M, 1], f32, tag="mx")
            nc.vector.tensor_reduce(mx[:], c_ps[:, :FEAT], axis=AX.X, op=ALU.max)
            nmx = small.tile([M, 1], f32, tag="nmx")
            nc.vector.tensor_scalar_mul(nmx[:], mx[:], -1.0)
            a_sb = small.tile([M, FEAT], f32, tag="a_sb")
            asum = small.tile([M, 1], f32, tag="asum")
            nc.scalar.activation(a_sb[:], c_ps[:, :FEAT], AF.Exp,
                                 bias=nmx[:, 0:1], accum_out=asum[:])
            arec = small.tile([M, 1], f32, tag="arec")
            nc.vector.reciprocal(arec[:], asum[:])
            ab = small.tile([M, FEAT], dt, tag="ab")
            nc.vector.tensor_scalar_mul(ab[:], a_sb[:], arec[:, 0:1])
            aT_ps = ps_tp.tile([FEAT, 128], dt, tag="tp", name="aT_ps")
            nc.tensor.transpose(aT_ps[:, :M], ab[:], idm[:M, :M])
            aT = small.tile([FEAT + 1, 128], dt, tag="aT")
            nc.vector.memset(aT[:], 1.0)
            nc.vector.tensor_copy(aT[:FEAT, :M], aT_ps[:, :M])

            gT_ps = ps_g.tile([128, nk_in * M], f32, tag="gt", name="gT_ps")
            for k in range(nk_in):
                nc.tensor.matmul(gT_ps[:, k * M:(k + 1) * M],
                                 w["W2"][:, k * 128:(k + 1) * 128],
                                 aT[:, :M], start=True, stop=True)
            hbT = (work if M == B else work1).tile(
                [128, nk_in, M], dt, tag="hbT%d" % M, name="hbT")
            ki = 0
            for seg, segnk in INseg:
                for k in range(segnk):
                    nc.vector.tensor_mul(hbT[:, ki, :], seg[:, k, :M],
                                         gT_ps[:, ki * M:(ki + 1) * M])
                    ki += 1

            y_ps = ps_mm.tile([M, d_in], f32, tag="mmout", name="y_ps")
            for t in range(d_in // 512):
                cs = slice(t * 512, (t + 1) * 512)
                ki = 0
                for seg, segnk in INseg:
                    for k in range(segnk):
                        nc.tensor.matmul(y_ps[:, cs], seg[:, k, :M],
                                         w["Wf"][:, ki, cs], start=(ki == 0),
                                         stop=False)
                        ki += 1
                for k in range(nk_in):
                    nc.tensor.matmul(y_ps[:, cs], hbT[:, k, :],
                                     w["Wf"][:, nk_in + k, cs],
                                     start=False, stop=False)
                nc.tensor.matmul(y_ps[:, cs], one_r[:, :M], w["Wfr"][:, cs],
                                 start=False, stop=True)
            yr = (work if M == B else work1).tile(
                [M, d_in], dt, tag="yr%d" % M, name="yr")
            nc.scalar.activation(yr[:], y_ps[:], AF.Relu)
            stats = small.tile([M, d_in // 512, 6], f32, tag="stats")
            for t in range(d_in // 512):
                nc.vector.bn_stats(stats[:, t, :], yr[:, t * 512:(t + 1) * 512])
            mv = small.tile([M, 2], f32, tag="mv")
            nc.vector.bn_aggr(mv[:], stats[:])
            sd = small.tile([M, 1], f32, tag="sd")
            nc.scalar.activation(sd[:], mv[:, 1:2], AF.Sqrt,
                                 bias=eps_sb[:M, 0:1])
            rstd = small.tile([M, 1], f32, tag="rstd")
            nc.vector.reciprocal(rstd[:], sd[:])
            pk = small.tile([M, 2], dt, tag="pk")
            nc.vector.tensor_copy(pk[:, 0:1], mv[:, 0:1])
            nc.vector.tensor_copy(pk[:, 1:2], sd[:])
            rows_ps = ps_tp.tile([2, 128], dt, tag="tp", name="rows_ps")
            nc.tensor.transpose(rows_ps[:, :M], pk[:], idm[:M, :M])
            rows = small.tile([2, 128], dt, tag="rows")
            nc.vector.tensor_copy(rows[:, :M], rows_ps[:, :M])
            yrT = (work if M == B else work1).tile(
                [128, nk_in, M], dt, tag="yrT%d" % M, name="yrT")
            transpose_in(yrT, yr[:], dt)
            return yrT, rows, rstd

        def big_head(w, INseg, M, rw_scale, func, out_sb):
            yrT, rows, rstd = head_fa(w, INseg, M)
            dout = w["dout"]
            nk = w["d_in"] // 128
            op = ps_no.tile([M, 512], f32, tag="nout", name="op")
            for k in range(nk):
                nc.tensor.matmul(op[:, :dout], yrT[:, k, :], w["Wn"][:, k, :],
                                 start=(k == 0), stop=False)
            nc.tensor.matmul(op[:, :dout], rows[:, :M], w["Wnr"][:],
                             start=False, stop=True)
            sc = small.tile([M, 1], f32, tag="sc")
            if rw_scale is None:
                nc.vector.tensor_copy(sc[:], rstd[:])
            else:
                nc.vector.tensor_scalar_mul(sc[:], rw_scale, rstd[:, 0:1])
            nc.scalar.activation(out_sb[:], op[:, :dout], func, scale=sc[:, 0:1])

        # ------------------ common: x, read-fa ------------------
        x_sb = wp.tile([B, IN_CH], f32, tag="x_msb")
        nc.sync.dma_start(out=x_sb[:], in_=x_d[:])
        xT = wp.tile([128, 4, B], f32, tag="xT")
        transpose_in(xT, x_sb[:], f32)
        xTb = wp.tile([128, 4, 2 * B], bf16, tag="xTb")
        for rep in range(2):
            nc.vector.tensor_copy(xTb[:, :, rep * B:(rep + 1) * B], xT[:])

        rfa = load_head("rfa", f32, rW1_d, rW2_d, rWf_d, None, IN_CH, 0)
        ryrT, rrows, rrstd = head_fa(rfa, [(xT[:], 4)], B)

        # ------------------ logit slices + stats ------------------
        ag1_in = dram.tile([B, F1], f32)
        ag1_out = dram.tile([NC * B, F1], f32)
        pay1 = work1.tile([B, F1], f32, tag="pay1")
        l_sb = wp.tile([B, RS * CW], f32, tag="l_sb")
        t0 = l_sb

        gb = small.tile([B, 1], f32, tag="gb")
        nc.gpsimd.dma_start(out=gb[:], in_=dap(consts_d, 0, [[0, B], [1, 1]]))
        msk_use = small.tile([16, 1], f32, tag="msk_use")
        nc.gpsimd.dma_start(out=msk_use[:],
                            in_=dap(consts_d, 1, [[0, 16], [1, 1]]))
        rrW = RS * CW
        NTW = [512, 512, CW - 1024]          # 1250 = 512+512+226
        for s in range(RS):
            for t in range(len(NTW)):
                wdt = NTW[t]
                c0 = s * CW + t * 512
                lt = ps_no.tile([B, 512], f32, tag="nout", name="lt")
                wk = sp.tile([128, 4, 512], f32, tag="wk")
                nc.sync.dma_start(
                    out=wk[:, :, :wdt],
                    in_=dap(rWs_d, c0, [[rrW, 128], [128 * rrW, 4], [1, wdt]]))
                wr = sp.tile([2, 512], f32, tag="wr")
                nc.sync.dma_start(
                    out=wr[:, :wdt],
                    in_=dap(rWs_d, IN_CH * rrW + c0, [[rrW, 2], [1, wdt]]))
                for k in range(4):
                    nc.tensor.matmul(lt[:, :wdt], ryrT[:, k, :],
                                     wk[:, k, :wdt], start=(k == 0), stop=False)
                nc.tensor.matmul(lt[:, :wdt], rrows[:, :B], wr[:, :wdt],
                                 start=False, stop=True)
                nc.scalar.activation(l_sb[:, c0:c0 + wdt], lt[:, :wdt],
                                     AF.Copy, scale=rrstd[:, 0:1])
            top8 = small.tile([B, 8], f32, tag="top8")
            nc.vector.max(top8[:], l_sb[:, s * CW:(s + 1) * CW])
            nc.vector.tensor_copy(pay1[:, s:s + 1], top8[:, 0:1])
            li = small.tile([B, 8], u32, tag="li")
            nc.vector.max_index(li[:], top8[:], l_sb[:, s * CW:(s + 1) * CW])
            lif = small.tile([B, 1], f32, tag="lif")
            nc.vector.tensor_copy(lif[:], li[:, 0:1])
            nc.vector.tensor_scalar_add(pay1[:, 6 + s:7 + s], lif[:],
                                        gb[:, 0:1])
            nm = small.tile([B, 1], f32, tag="nm")
            nc.vector.tensor_scalar_mul(nm[:], top8[:, 0:1], -1.0)
            zc = small.tile([B, 1], f32, tag="zc")
            nc.scalar.activation(t0[:, s * CW:(s + 1) * CW],
                                 l_sb[:, s * CW:(s + 1) * CW], AF.Exp,
                                 bias=nm[:, 0:1], accum_out=zc[:])
            nc.vector.tensor_copy(pay1[:, 3 + s:4 + s], zc[:])
        nc.vector.memset(pay1[:, 9:F1], 0.0)
        nc.sync.dma_start(out=ag1_in[:], in_=pay1[:])
        nc.gpsimd.collective_compute(
            "AllGather", ALU.bypass, replica_groups=[list(range(NC))],
            ins=[ag1_in.opt()], outs=[ag1_out.opt()])

        # ------------------ combine stats ------------------
        comb = work1.tile([B, NC, F1], f32, tag="comb")
        nc.sync.dma_start(
            out=comb[:], in_=dap(ag1_out, 0, [[F1, B], [B * F1, NC], [1, F1]]))
        Mg = small.tile([B, 1], f32, tag="Mg")
        nc.vector.tensor_reduce(Mg[:], comb[:, :, 0:3], axis=AX.XY, op=ALU.max)
        nMg = small.tile([B, 1], f32, tag="nMg")
        nc.vector.tensor_scalar_mul(nMg[:], Mg[:], -1.0)
        et = work1.tile([B, NC, 3], f32, tag="et")
        nc.scalar.activation(et[:], comb[:, :, 0:3], AF.Exp, bias=nMg[:, 0:1])
        nc.vector.tensor_mul(et[:], et[:], comb[:, :, 3:6])
        Zg = small.tile([B, 1], f32, tag="Zg")
        nc.vector.tensor_reduce(Zg[:], et[:], axis=AX.XY, op=ALU.add)
        Zrec = small.tile([B, 1], f32, tag="Zrec")
        nc.vector.reciprocal(Zrec[:], Zg[:])
        combA = comb[:]
        Lmax = small.tile([B, RS], f32, tag="Lmax")
        nc.vector.tensor_reduce(
            Lmax[:],
            bass.AP(tensor=combA.tensor, offset=combA.offset,
                    ap=[combA.ap[0], [1, RS], [F1, NC]]),
            axis=AX.X, op=ALU.max)
        pmax = small.tile([B, RS], f32, tag="pmax")
        nc.scalar.activation(pmax[:], Lmax[:], AF.Exp, bias=nMg[:, 0:1])
        nc.vector.tensor_scalar_mul(pmax[:], pmax[:], Zrec[:, 0:1])
        npmax = small.tile([B, RS], f32, tag="npmax")
        nc.vector.tensor_scalar_mul(npmax[:], pmax[:], -1.0)
        alph = small.tile([B, RS], f32, tag="alph")
        nc.scalar.activation(alph[:], pay1[:, 0:3], AF.Exp, bias=nMg[:, 0:1])
        nc.vector.tensor_scalar_mul(alph[:], alph[:], Zrec[:, 0:1])
        G = small.tile([B, RS], f32, tag="G")
        for s in range(RS):
            mskr = small.tile([B, NC], f32, tag="mskr")
            nc.vector.tensor_scalar(
                out=mskr[:],
                in0=bass.AP(tensor=combA.tensor, offset=combA.offset + s,
                            ap=[combA.ap[0], [F1, NC]]),
                scalar1=Lmax[:, s:s + 1], scalar2=None, op0=ALU.is_equal)
            nc.vector.tensor_mul(
                mskr[:], mskr[:],
                bass.AP(tensor=combA.tensor, offset=combA.offset + 6 + s,
                        ap=[combA.ap[0], [F1, NC]]))
            nc.vector.tensor_reduce(G[:, s:s + 1], mskr[:], axis=AX.X,
                                    op=ALU.add)

        # ------------------ pass2 + AG2 ------------------
        ag2_in = dram.tile([B, F2], f32)
        ag2_out = dram.tile([NC * B, F2], f32)
        pay2 = work1.tile([B, F2], f32, tag="pay2")
        nc.vector.memset(pay2[:], 0.0)
        for s in range(RS):
            q = small.tile([B, 1], f32, tag="q")
            junk = sp.tile([B, CW], f32, tag="wk", name="junk")
            nc.scalar.activation(junk[:], t0[:, s * CW:(s + 1) * CW],
                                 AF.Exp, bias=npmax[:, s:s + 1],
                                 scale=alph[:, s:s + 1], accum_out=q[:])
            nc.vector.tensor_copy(pay2[:, s:s + 1], q[:])
        nc.sync.dma_start(out=ag2_in[:], in_=pay2[:])
        nc.gpsimd.collective_compute(
            "AllGather", ALU.bypass, replica_groups=[list(range(NC))],
            ins=[ag2_in.opt()], outs=[ag2_out.opt()])
        q2 = work1.tile([B, NC, F2], f32, tag="q2")
        nc.sync.dma_start(
            out=q2[:], in_=dap(ag2_out, 0, [[F2, B], [B * F2, NC], [1, F2]]))
        q2b = q2[:]
        qs = small.tile([B, RS], f32, tag="qs")
        nc.vector.tensor_reduce(
            qs[:],
            bass.AP(tensor=q2b.tensor, offset=q2b.offset,
                    ap=[q2b.ap[0], [1, RS], [F2, NC]]),
            axis=AX.X, op=ALU.add)
        rw = small.tile([B, RS], f32, tag="rw")
        nc.vector.reciprocal(rw[:], qs[:])

        # ------------------ gather (uniform; idx select by mask) ----------
        Gsc = dram.tile([B, RS], f32)
        nc.sync.dma_start(out=Gsc[:], in_=G[:])
        g16 = work1.tile([16, 12], f32, tag="g16")
        nc.sync.dma_start(
            out=g16[:], in_=dap(Gsc, 0, [[RS, 16], [1, RS], [16 * RS, 4]]))
        r16 = work1.tile([16, 12], f32, tag="r16")
        nc.sync.dma_start(out=r16[:], in_=ridx16_d[:])
        dif = small.tile([16, 12], f32, tag="dif")
        nc.vector.tensor_sub(dif[:], g16[:], r16[:])
        sel = small.tile([16, 12], f32, tag="sel")
        nc.vector.scalar_tensor_tensor(out=sel[:], in0=dif[:],
                                       scalar=msk_use[:, 0:1], in1=r16[:],
                                       op0=ALU.mult, op1=ALU.add)
        # exact f32 -> i16: add 2^23 so the integer sits in the low mantissa
        # bits, then take the low half of each f32 word.
        sel2 = small.tile([16, 12], f32, tag="sel2")
        nc.vector.tensor_scalar_add(sel2[:], sel[:], 8388608.0)
        selb = sel2[:].bitcast(i16).rearrange("p (c two) -> p c two", two=2)
        idxs = work1.tile([128, 12], i16, tag="idxs")
        nc.vector.memset(idxs[:], 0)
        nc.vector.tensor_copy(idxs[:16, :], selb[:, :, 0])
        # hw requires the 16-partition index block replicated across all
        # 8 gpsimd cores' stripes
        for kk in range(1, 8):
            nc.gpsimd.dma_start(out=idxs[16 * kk:16 * (kk + 1), :],
                                in_=idxs[0:16, :])
        gath = sp.tile([128, 2, SLOT], f32, tag="wk", name="gath")
        nc.gpsimd.dma_gather(gath[:], memory_d[:], idxs[:],
                             RS * B, RS * B, SLOT)
        gathb = work1.tile([128, 2, SLOT], bf16, tag="gathb")
        nc.vector.tensor_copy(gathb[:].rearrange("p a c -> p (a c)"),
                              gath[:].rearrange("p a c -> p (a c)"))

        # ------------------ update heads (uniform) ------------------
        wU = load_head("bigA", bf16, bigd["W1a"], bigd["W2a"], bigd["Wfa"],
                       bigd["Wna"], 2 * SLOT, SLOT)
        wS = load_head("sm", bf16, sW1_d, sW2_d, sWf_d, sWn_d, IN_CH, RS)
        uw_sb = small.tile([B, RS], f32, tag="uw_sb")
        big_head(wS, [(xTb[:], 4)], B, None, AF.Sigmoid, uw_sb[:])

        pay3a = work1.tile([128, 4, B], bf16, tag="pay3a")
        pay3b = work1.tile([128, 8, B], bf16, tag="pay3b")

        def mix_and_pack(dst, u_ap, mem_bm, uwcol, koff):
            bp = u_ap.base_partition()
            M = u_ap.shape[0]
            d_t = work1.tile([128, SLOT], f32, tag="dmix")
            d = d_t[bp:bp + M, :]
            nc.vector.tensor_sub(d, u_ap, mem_bm)
            r_t = work1.tile([128, SLOT], f32, tag="rmix")
            r = r_t[bp:bp + M, :]
            nc.vector.scalar_tensor_tensor(out=r, in0=d, scalar=uwcol,
                                           in1=mem_bm, op0=ALU.mult,
                                           op1=ALU.add)
            rb_t = work1.tile([128, SLOT], bf16, tag="rbmix")
            rb = rb_t[bp:bp + M, :]
            nc.vector.tensor_copy(rb, r)
            transpose_in(dst, rb, bf16, koff=koff)

        # u_0 -> pay3a
        m0T = work1.tile([128, 4, B], bf16, tag="m0T")
        transpose_in(m0T, gathb[0:B, 0, :], bf16)
        u0 = work1.tile([B, SLOT], f32, tag="u0")
        big_head(wU, [(m0T[:], 4), (xTb[:], 4)], B, None, AF.Relu, u0[:])
        mix_and_pack(pay3a, u0[:], gathb[0:B, 0, :], uw_sb[:, 0:1], 0)

        ag3a_in = dram.tile([128, 4 * B], bf16)
        ag3a_out = dram.tile([NC * 128, 4 * B], bf16)
        nc.sync.dma_start(out=ag3a_in[:],
                          in_=pay3a[:].rearrange("p a b -> p (a b)"))
        nc.gpsimd.collective_compute(
            "AllGather", ALU.bypass, replica_groups=[list(range(NC))],
            ins=[ag3a_in.opt()], outs=[ag3a_out.opt()])

        # u_1, u_2 (M=128) -> pay3b; col 0:64 = s2 (base 0), 64:128 = s1
        m12T = work1.tile([128, 4, 2 * B], bf16, tag="m12T")
        transpose_in(m12T, gathb[0:B, 1, :], bf16, coff=0)
        transpose_in(m12T, gathb[B:2 * B, 0, :], bf16, coff=B)
        u12 = work1.tile([2 * B, SLOT], f32, tag="u12")
        big_head(wU, [(m12T[:], 4), (xTb[:], 4)], 2 * B, None, AF.Relu, u12[:])
        mix_and_pack(pay3b, u12[0:B, :], gathb[0:B, 1, :],
                     uw_sb[:, 2:3], 4)
        uwsh = work1.tile([128, 1], f32, tag="uwsh")
        nc.sync.dma_start(out=uwsh[64:128, :], in_=uw_sb[:, 1:2])
        mix_and_pack(pay3b, u12[B:2 * B, :], gathb[B:2 * B, 0, :],
                     uwsh[64:128, 0:1], 0)

        ag3b_in = dram.tile([128, 8 * B], bf16)
        ag3b_out = dram.tile([NC * 128, 8 * B], bf16)
        nc.sync.dma_start(out=ag3b_in[:],
                          in_=pay3b[:].rearrange("p a b -> p (a b)"))
        nc.gpsimd.collective_compute(
            "AllGather", ALU.bypass, replica_groups=[list(range(NC))],
            ins=[ag3b_in.opt()], outs=[ag3b_out.opt()])

        # ------------------ chain (uniform; real only on core 7) ----------
        wB_h = load_head("bigB", bf16, bigd["W1b"], bigd["W2b"], bigd["Wfb"],
                         bigd["Wnb"], 2 * SLOT, SLOT)
        rT = work1.tile([128, 12, B], bf16, tag="rT")
        nc.sync.dma_start(
            out=rT[:, 0:4, :].rearrange("p a b -> p (a b)"),
            in_=ag3a_out[U_CORE * 128:(U_CORE + 1) * 128, :])
        nc.sync.dma_start(
            out=rT[:, 4:12, :].rearrange("p a b -> p (a b)"),
            in_=ag3b_out[U_CORE * 128:(U_CORE + 1) * 128, :])
        r2T = work1.tile([128, 12, B], bf16, tag="r2T")
        nc.sync.dma_start(
            out=r2T[:, 0:4, :].rearrange("p a b -> p (a b)"),
            in_=ag3a_out[U2_CORE * 128:(U2_CORE + 1) * 128, :])
        nc.sync.dma_start(
            out=r2T[:, 4:12, :].rearrange("p a b -> p (a b)"),
            in_=ag3b_out[U2_CORE * 128:(U2_CORE + 1) * 128, :])
        m_sb = wp.tile([B, SLOT], f32, tag="x_msb", name="m_sb")
        nc.vector.memset(m_sb[:], 0.0)
        mT = wp.tile([128, 4, B], bf16, tag="mT")
        nc.vector.memset(mT[:].rearrange("p a b -> p (a b)"), 0.0)
        for step in range(6):
            wH = wU if step % 2 == 0 else wB_h
            s = step // 2
            src = rT if step % 2 == 0 else r2T
            seg = [(src[:, s * 4:(s + 1) * 4, :], 4), (mT[:], 4)]
            t_sb = work1.tile([B, SLOT], f32, tag="t_sb")
            big_head(wH, seg, B, rw[:, s:s + 1], AF.Relu, t_sb[:])
            nc.vector.tensor_add(m_sb[:], m_sb[:], t_sb[:])
            if step < 5:
                mb = work1.tile([B, SLOT], bf16, tag="rbmix", name="mb")
                nc.vector.tensor_copy(mb[:], m_sb[:])
                transpose_in(mT, mb[:], bf16)
        nc.sync.dma_start(out=out_d[:], in_=m_sb[:])

        dbg = work1.tile([B, 16], f32, tag="dbg")
        nc.vector.memset(dbg[:], 0.0)
        nc.vector.tensor_copy(dbg[:, 0:3], G[:])
        nc.vector.tensor_copy(dbg[:, 3:6], rw[:])
        nc.vector.tensor_copy(dbg[:, 6:9], pmax[:])
        nc.vector.tensor_copy(dbg[:, 9:10], Mg[:])
        nc.vector.tensor_copy(dbg[:, 10:11], Zg[:])
        nc.sync.dma_start(out=dbg_d[:], in_=dbg[:])

    return nc


def kernel(inputs, memory, random_indices, params):
    from concourse.bass_utils import run_bass_kernel_spmd
    in_maps = _build_inputs(inputs, memory, random_indices, params)
    nc = _build_nc()
    if not nc.is_finalized():
        nc.finalize()
    res = run_bass_kernel_spmd(nc, in_maps, core_ids=list(range(NC)),
                               trace=bool(int(os.environ.get("KTRACE", "0"))))
    kernel.last_results = res
    return res.results[CH_CORE]["out"].copy()


def kernel_timed(inputs, memory, random_indices, params, iters=3):
    """Run once for outputs, then time warm device executions (device-
    resident inputs, donated outputs). Returns (out, best_exec_seconds)."""
    import time
    import jax
    import numpy as np
    from jax.sharding import Mesh, PartitionSpec
    from jax.experimental.shard_map import shard_map
    import concourse.mybir as mybir
    from concourse.bass2jax import (_bass_exec_p, partition_id_tensor,
                                    install_neuronx_cc_hook)

    in_maps = _build_inputs(inputs, memory, random_indices, params)
    nc = _build_nc()
    if not nc.is_finalized():
        nc.finalize()
    install_neuronx_cc_hook()
    partition_name = nc.partition_id_tensor.name if nc.partition_id_tensor         else None
    in_names, out_names, out_avals, zero_outs = [], [], [], []
    for alloc in nc.m.functions[0].allocations:
        if not isinstance(alloc, mybir.MemoryLocationSet):
            continue
        name = alloc.memorylocations[0].name
        if alloc.kind == "ExternalInput":
            if name != partition_name:
                in_names.append(name)
        elif alloc.kind == "ExternalOutput":
            shape = tuple(alloc.tensor_shape)
            dtype = mybir.dt.np(alloc.dtype)
            out_names.append(name)
            out_avals.append(jax.core.ShapedArray(shape, dtype))
            zero_outs.append(np.zeros(shape, dtype))
    n_params = len(in_names)
    n_outs = len(out_avals)
    all_names = list(in_names) + list(out_names)
    if partition_name is not None:
        all_names.append(partition_name)
    donate = tuple(range(n_params, n_params + n_outs))

    def _body(*args):
        operands = list(args)
        if partition_name is not None:
            operands.append(partition_id_tensor())
        return tuple(_bass_exec_p.bind(
            *operands, out_avals=tuple(out_avals), in_names=tuple(all_names),
            out_names=tuple(out_names), lowering_input_output_aliases=(),
            sim_require_finite=True, sim_require_nnan=True, nc=nc))

    devices = jax.devices()[:NC]
    mesh = Mesh(np.asarray(devices), ("core",))
    sharded = jax.jit(
        shard_map(_body, mesh=mesh,
                  in_specs=(PartitionSpec("core"),) * (n_params + n_outs),
                  out_specs=(PartitionSpec("core"),) * n_outs,
                  check_rep=False),
        donate_argnums=donate, keep_unused=True)
    concat_in = [np.concatenate([np.asarray(in_maps[c][in_names[i]])
                                 for c in range(NC)], axis=0)
                 for i in range(n_params)]
    sh = jax.sharding.NamedSharding(mesh, PartitionSpec("core"))
    dev_in = [jax.device_put(a, sh) for a in concat_in]

    def zeros():
        return [jax.device_put(
            np.zeros((NC * z.shape[0], *z.shape[1:]), z.dtype), sh)
            for z in zero_outs]

    out_arrs = jax.block_until_ready(sharded(*dev_in, *zeros()))
    out = np.asarray(out_arrs[out_names.index("out")]).reshape(
        NC, B, SLOT)[CH_CORE]
    best = None
    for _ in range(iters):
        zo = zeros()
        jax.block_until_ready(zo)
        t0 = time.perf_counter()
        r = sharded(*dev_in, *zo)
        jax.block_until_ready(r)
        dt = time.perf_counter() - t0
        best = dt if best is None else min(best, dt)
    return out.copy(), best


# revision 21
# speedup vs baseline: 1.4709x; 1.4709x over previous
"""Trainium2 Bass kernel for nn_MemoryAccess (scatter_memory).

Uniform SPMD program on 8 cores (no control flow); per-core behavior is
driven purely by per-core input data:
  - every core holds a 1250-column slice (x3 s-blocks) of the read
    projection (fp32) and computes logits + softmax stats (max / sumexp /
    argmax) for its slice.
  - every core mirrors the update-head + chain code on its own weight
    slots; the slots hold real weights only on their owner core
    (core 5: upd_mem + upd_w, core 6: upd_mem_rand + upd_w_rand,
    core 7: apply_mem + apply_mem_rand), zeros elsewhere.
  - gather indices select between the computed argmax (core 5, mask=1)
    and the host-prepacked random indices (all other cores, mask=0).
  - 4 small AllGathers: softmax stats, q partials (2nd softmax), r/r2
    vectors for s=0, then s=1,2. Core 7 runs the serial 6-step m chain.
LayerNorm in every head is folded into the following matmul: two extra
k-rows (mu x -colsum(W'), sqrt(var+eps) x c0-row) plus a per-row rstd
scale fused into the Relu/Sigmoid activation (relu(s*x)=s*relu(x), s>0).
"""

import os

os.environ.setdefault("JAX_PLATFORMS", "cpu")

import numpy as np
import ml_dtypes

IN_CH = 512
SLOT = 512
FEAT = 16
AVAIL = 10000
RS = 3
B = 64
NC = 8

CW = AVAIL // NC        # 1250 cols per (core, s-block)
U_CORE = 5              # upd_mem + upd_w owner (uses computed argmax gather)
U2_CORE = 6             # upd_mem_rand + upd_w_rand owner (random gather)
CH_CORE = 7             # apply_mem + apply_mem_rand owner; runs the chain

F32 = np.float32
BF16 = ml_dtypes.bfloat16
F1 = 12   # AG1 cols: m_chunk[3] z_chunk[3] gidx[3] pad[3]
F2 = 4    # AG2 cols: q[3] pad


def _pack_head(p, dtype):
    fa = p["fa"]
    W1 = np.concatenate([np.asarray(fa["W1"], F32),
                         np.asarray(fa["b1"], F32)[None, :]], 0)
    W2 = np.concatenate([np.asarray(fa["W2"], F32),
                         np.asarray(fa["b2"], F32)[None, :]], 0)
    Wf = np.concatenate([np.asarray(fa["Wf"], F32),
                         np.asarray(fa["bf"], F32)[None, :]], 0)
    Wp = np.asarray(fa["ln_g"], F32)[:, None] * np.asarray(p["W"], F32)
    c0 = (np.asarray(fa["ln_b"], F32) @ np.asarray(p["W"], F32)
          + np.asarray(p["b"], F32))
    Wn = np.concatenate([Wp, -Wp.sum(0)[None, :], c0[None, :]], 0)
    return [np.ascontiguousarray(a.astype(dtype)) for a in (W1, W2, Wf, Wn)]


def _idx16(idx):
    """(192,) row indices -> (16,12) gather layout (flat i -> [i%16, i//16])."""
    a = np.zeros((16, 12), F32)
    for i in range(192):
        a[i % 16, i // 16] = float(idx[i])
    return a


def _build_inputs(inputs, memory, random_indices, params):
    x = np.ascontiguousarray(np.asarray(inputs, F32))
    mem = np.ascontiguousarray(np.asarray(memory, F32))
    ridx = np.asarray(random_indices)
    ridx16 = _idx16(ridx.T.reshape(-1))         # order s*64+b

    rH = _pack_head(params["read_w"], F32)
    uwH = _pack_head(params["upd_w"], BF16)
    uwrH = _pack_head(params["upd_w_rand"], BF16)
    uH = _pack_head(params["upd_mem"], BF16)
    u2H = _pack_head(params["upd_mem_rand"], BF16)
    apH = _pack_head(params["apply_mem"], BF16)
    aprH = _pack_head(params["apply_mem_rand"], BF16)
    zH_big = [np.zeros_like(a) for a in uH]
    zH_sm = [np.zeros_like(a) for a in uwH]

    in_maps = []
    for c in range(NC):
        d = {"x": x, "rW1": rH[0], "rW2": rH[1], "rWf": rH[2]}
        sl = np.empty((IN_CH + 2, RS * CW), F32)
        for s in range(RS):
            sl[:, s * CW:(s + 1) * CW] = \
                rH[3][:, s * AVAIL + c * CW:s * AVAIL + (c + 1) * CW]
        d["rWslice"] = np.ascontiguousarray(sl)
        bigA = {U_CORE: uH, U2_CORE: u2H, CH_CORE: apH}.get(c, zH_big)
        bigB = aprH if c == CH_CORE else zH_big
        sm = {U_CORE: uwH, U2_CORE: uwrH}.get(c, zH_sm)
        for n, a in zip(("bW1a", "bW2a", "bWfa", "bWna"), bigA):
            d[n] = a
        for n, a in zip(("bW1b", "bW2b", "bWfb", "bWnb"), bigB):
            d[n] = a
        for n, a in zip(("sW1", "sW2", "sWf", "sWn"), sm):
            d[n] = a
        d["memory"] = mem if c in (U_CORE, U2_CORE) else \
            np.zeros((AVAIL, SLOT), F32)
        d["ridx16"] = ridx16
        d["consts"] = np.array([[float(c * CW),
                                 1.0 if c == U_CORE else 0.0, 0.0, 0.0]], F32)
        in_maps.append(d)
    return in_maps


def _build_nc():
    import concourse.bass as bass
    import concourse.mybir as mybir
    import concourse.tile as tile
    from concourse import bacc
    from concourse.masks import make_identity

    f32 = mybir.dt.float32
    bf16 = mybir.dt.bfloat16
    i16 = mybir.dt.int16
    u32 = mybir.dt.uint32
    AF = mybir.ActivationFunctionType
    ALU = mybir.AluOpType
    AX = mybir.AxisListType

    nc = bacc.Bacc(None, target_bir_lowering=False)

    def din(name, shape, dt=f32):
        return nc.dram_tensor(name, shape, dt, kind="ExternalInput")

    x_d = din("x", [B, IN_CH])
    rW1_d = din("rW1", [IN_CH + 1, FEAT])
    rW2_d = din("rW2", [FEAT + 1, IN_CH])
    rWf_d = din("rWf", [2 * IN_CH + 1, IN_CH])
    rWs_d = din("rWslice", [IN_CH + 2, RS * CW])
    bigd = {}
    for sl in "ab":
        bigd["W1" + sl] = din("bW1" + sl, [2 * SLOT + 1, FEAT], bf16)
        bigd["W2" + sl] = din("bW2" + sl, [FEAT + 1, 2 * SLOT], bf16)
        bigd["Wf" + sl] = din("bWf" + sl, [4 * SLOT + 1, 2 * SLOT], bf16)
        bigd["Wn" + sl] = din("bWn" + sl, [2 * SLOT + 2, SLOT], bf16)
    sW1_d = din("sW1", [IN_CH + 1, FEAT], bf16)
    sW2_d = din("sW2", [FEAT + 1, IN_CH], bf16)
    sWf_d = din("sWf", [2 * IN_CH + 1, IN_CH], bf16)
    sWn_d = din("sWn", [IN_CH + 2, RS], bf16)
    memory_d = din("memory", [AVAIL, SLOT])
    ridx16_d = din("ridx16", [16, 12])
    consts_d = din("consts", [1, 4])
    out_d = nc.dram_tensor("out", [B, SLOT], f32, kind="ExternalOutput")

    dbg_d = nc.dram_tensor("dbg", [B, 16], f32, kind="ExternalOutput")

    def dap(handle_or_tile, offset, ap):
        base = handle_or_tile[:]
        return bass.AP(tensor=base.tensor, offset=base.offset + offset, ap=ap)

    import contextlib

    with tile.TileContext(nc) as tc, contextlib.ExitStack() as ctx:
        wp = ctx.enter_context(tc.tile_pool(name="wp", bufs=1))
        sp = ctx.enter_context(tc.tile_pool(name="sp", bufs=2))
        work = ctx.enter_context(tc.tile_pool(name="work", bufs=2))
        small = ctx.enter_context(tc.tile_pool(name="small", bufs=2))
        ps_mm = ctx.enter_context(tc.tile_pool(name="ps_mm", bufs=1, space="PSUM"))
        ps_no = ctx.enter_context(tc.tile_pool(name="ps_no", bufs=2, space="PSUM"))
        ps_g = ctx.enter_context(tc.tile_pool(name="ps_g", bufs=1, space="PSUM"))
        ps_tp = ctx.enter_context(tc.tile_pool(name="ps_tp", bufs=2, space="PSUM"))
        dram = ctx.enter_context(tc.tile_pool(name="dram", bufs=1, space="DRAM"))
        work1 = ctx.enter_context(tc.tile_pool(name="work1", bufs=1))

        ident = wp.tile([128, 128], f32, tag="ident")
        make_identity(nc, ident[:])
        identb = wp.tile([128, 128], bf16, tag="identb")
        nc.vector.tensor_copy(identb[:], ident[:])
        ones = wp.tile([1, 2 * B], f32, tag="ones")
        nc.vector.memset(ones[:], 1.0)
        onesb = wp.tile([1, 2 * B], bf16, tag="onesb")
        nc.vector.memset(onesb[:], 1.0)
        eps_sb = wp.tile([128, 1], f32, tag="eps")
        nc.vector.memset(eps_sb[:], 1e-5)

        def transpose_in(dst, src_ap, dt, koff=0, coff=0):
            M = src_ap.shape[0]
            nk = src_ap.shape[-1] // 128
            src2 = src_ap.rearrange("m (k p) -> m k p", p=128)
            idm = ident if dt == f32 else identb
            bp = src_ap.base_partition()
            for k in range(nk):
                pt = ps_tp.tile([128, 128], dt, tag="tp")
                nc.tensor.transpose(pt[:, :M], src2[:, k, :],
                                    idm[bp:bp + M, bp:bp + M])
                nc.vector.tensor_copy(dst[:, koff + k, coff:coff + M], pt[:, :M])

        def load_head(tag, dt, W1d, W2d, Wfd, Wnd, d_in, dout):
            nk = d_in // 128
            w = {}
            w["W1"] = wp.tile([128, nk, FEAT], dt, tag=tag + "W1",
                              name=tag + "W1")
            nc.sync.dma_start(
                out=w["W1"][:],
                in_=dap(W1d, 0, [[FEAT, 128], [128 * FEAT, nk], [1, FEAT]]))
            w["W1r"] = wp.tile([1, FEAT], dt, tag=tag + "W1r",
                               name=tag + "W1r")
            nc.sync.dma_start(out=w["W1r"][:], in_=W1d[d_in:d_in + 1, :])
            w["W2"] = wp.tile([FEAT + 1, d_in], dt, tag=tag + "W2",
                              name=tag + "W2")
            nc.sync.dma_start(out=w["W2"][:], in_=W2d[:])
            wftag = "bigAWf" if tag in ("rfa", "bigA") else tag + "Wf"
            w["Wf"] = wp.tile([128, 2 * nk, d_in], dt, tag=wftag,
                              name=tag + "Wf")
            nc.sync.dma_start(
                out=w["Wf"][:],
                in_=dap(Wfd, 0, [[d_in, 128], [128 * d_in, 2 * nk], [1, d_in]]))
            w["Wfr"] = wp.tile([1, d_in], dt, tag=tag + "Wfr",
                               name=tag + "Wfr")
            nc.sync.dma_start(out=w["Wfr"][:], in_=Wfd[2 * d_in:2 * d_in + 1, :])
            if Wnd is not None:
                w["Wn"] = wp.tile([128, nk, dout], dt, tag=tag + "Wn",
                                  name=tag + "Wn")
                nc.sync.dma_start(
                    out=w["Wn"][:],
                    in_=dap(Wnd, 0, [[dout, 128], [128 * dout, nk], [1, dout]]))
                w["Wnr"] = wp.tile([2, dout], dt, tag=tag + "Wnr",
                                   name=tag + "Wnr")
                nc.sync.dma_start(out=w["Wnr"][:], in_=Wnd[d_in:d_in + 2, :])
            w["d_in"], w["dout"], w["dt"] = d_in, dout, dt
            return w

        def head_fa(w, INseg, M):
            dt = w["dt"]
            d_in = w["d_in"]
            nk_in = d_in // 128
            one_r = ones if dt == f32 else onesb
            idm = ident if dt == f32 else identb

            c_ps = ps_no.tile([M, 512], f32, tag="nout", name="c_ps")
            ki = 0
            for seg, segnk in INseg:
                for k in range(segnk):
                    nc.tensor.matmul(c_ps[:, :FEAT], seg[:, k, :M],
                                     w["W1"][:, ki, :], start=(ki == 0),
                                     stop=False)
                    ki += 1
            nc.tensor.matmul(c_ps[:, :FEAT], one_r[:, :M], w["W1r"][:],
                             start=False, stop=True)
            mx = small.tile([M, 1], f32, tag="mx")
            nc.vector.tensor_reduce(mx[:], c_ps[:, :FEAT], axis=AX.X, op=ALU.max)
            nmx = small.tile([M, 1], f32, tag="nmx")
            nc.vector.tensor_scalar_mul(nmx[:], mx[:], -1.0)
            a_sb = small.tile([M, FEAT], f32, tag="a_sb")
            asum = small.tile([M, 1], f32, tag="asum")
            nc.scalar.activation(a_sb[:], c_ps[:, :FEAT], AF.Exp,
                                 bias=nmx[:, 0:1], accum_out=asum[:])
            arec = small.tile([M, 1], f32, tag="arec")
            nc.vector.reciprocal(arec[:], asum[:])
            ab = small.tile([M, FEAT], dt, tag="ab")
            nc.vector.tensor_scalar_mul(ab[:], a_sb[:], arec[:, 0:1])
            aT_ps = ps_tp.tile([FEAT, 128], dt, tag="tp", name="aT_ps")
            nc.tensor.transpose(aT_ps[:, :M], ab[:], idm[:M, :M])
            aT = small.tile([FEAT + 1, 128], dt, tag="aT")
            nc.vector.memset(aT[:], 1.0)
            nc.vector.tensor_copy(aT[:FEAT, :M], aT_ps[:, :M])

            gT_ps = ps_g.tile([128, nk_in * M], f32, tag="gt", name="gT_ps")
            for k in range(nk_in):
                nc.tensor.matmul(gT_ps[:, k * M:(k + 1) * M],
                                 w["W2"][:, k * 128:(k + 1) * 128],
                                 aT[:, :M], start=True, stop=True)
            hbT = (work if M == B else work1).tile(
                [128, nk_in, M], dt, tag="hbT%d" % M, name="hbT")
            ki = 0
            for seg, segnk in INseg:
                for k in range(segnk):
                    nc.vector.tensor_mul(hbT[:, ki, :], seg[:, k, :M],
                                         gT_ps[:, ki * M:(ki + 1) * M])
                    ki += 1

            y_ps = ps_mm.tile([M, d_in], f32, tag="mmout", name="y_ps")
            for t in range(d_in // 512):
                cs = slice(t * 512, (t + 1) * 512)
                ki = 0
                for seg, segnk in INseg:
                    for k in range(segnk):
                        nc.tensor.matmul(y_ps[:, cs], seg[:, k, :M],
                                         w["Wf"][:, ki, cs], start=(ki == 0),
                                         stop=False)
                        ki += 1
                for k in range(nk_in):
                    nc.tensor.matmul(y_ps[:, cs], hbT[:, k, :],
                                     w["Wf"][:, nk_in + k, cs],
                                     start=False, stop=False)
                nc.tensor.matmul(y_ps[:, cs], one_r[:, :M], w["Wfr"][:, cs],
                                 start=False, stop=True)
            yr = (work if M == B else work1).tile(
                [M, d_in], dt, tag="yr%d" % M, name="yr")
            nc.scalar.activation(yr[:], y_ps[:], AF.Relu)
            stats = small.tile([M, d_in // 512, 6], f32, tag="stats")
            for t in range(d_in // 512):
                nc.vector.bn_stats(stats[:, t, :], yr[:, t * 512:(t + 1) * 512])
            mv = small.tile([M, 2], f32, tag="mv")
            nc.vector.bn_aggr(mv[:], stats[:])
            sd = small.tile([M, 1], f32, tag="sd")
            nc.scalar.activation(sd[:], mv[:, 1:2], AF.Sqrt,
                                 bias=eps_sb[:M, 0:1])
            rstd = small.tile([M, 1], f32, tag="rstd")
            nc.vector.reciprocal(rstd[:], sd[:])
            pk = small.tile([M, 2], dt, tag="pk")
            nc.vector.tensor_copy(pk[:, 0:1], mv[:, 0:1])
            nc.vector.tensor_copy(pk[:, 1:2], sd[:])
            rows_ps = ps_tp.tile([2, 128], dt, tag="tp", name="rows_ps")
            nc.tensor.transpose(rows_ps[:, :M], pk[:], idm[:M, :M])
            rows = small.tile([2, 128], dt, tag="rows")
            nc.vector.tensor_copy(rows[:, :M], rows_ps[:, :M])
            yrT = (work if M == B else work1).tile(
                [128, nk_in, M], dt, tag="yrT%d" % M, name="yrT")
            transpose_in(yrT, yr[:], dt)
            return yrT, rows, rstd

        def big_head(w, INseg, M, rw_scale, func, out_sb):
            yrT, rows, rstd = head_fa(w, INseg, M)
            dout = w["dout"]
            nk = w["d_in"] // 128
            op = ps_no.tile([M, 512], f32, tag="nout", name="op")
            for k in range(nk):
                nc.tensor.matmul(op[:, :dout], yrT[:, k, :], w["Wn"][:, k, :],
                                 start=(k == 0), stop=False)
            nc.tensor.matmul(op[:, :dout], rows[:, :M], w["Wnr"][:],
                             start=False, stop=True)
            sc = small.tile([M, 1], f32, tag="sc")
            if rw_scale is None:
                nc.vector.tensor_copy(sc[:], rstd[:])
            else:
                nc.vector.tensor_scalar_mul(sc[:], rw_scale, rstd[:, 0:1])
            nc.scalar.activation(out_sb[:], op[:, :dout], func, scale=sc[:, 0:1])

        # ------------------ common: x, read-fa ------------------
        x_sb = wp.tile([B, IN_CH], f32, tag="x_msb")
        nc.sync.dma_start(out=x_sb[:], in_=x_d[:])
        xT = wp.tile([128, 4, B], f32, tag="xT")
        transpose_in(xT, x_sb[:], f32)
        xTb = wp.tile([128, 4, 2 * B], bf16, tag="xTb")
        for rep in range(2):
            nc.vector.tensor_copy(xTb[:, :, rep * B:(rep + 1) * B], xT[:])

        rfa = load_head("rfa", f32, rW1_d, rW2_d, rWf_d, None, IN_CH, 0)
        ryrT, rrows, rrstd = head_fa(rfa, [(xT[:], 4)], B)

        # ------------------ logit slices + stats ------------------
        ag1_in = dram.tile([B, F1], f32)
        ag1_out = dram.tile([NC * B, F1], f32)
        pay1 = work1.tile([B, F1], f32, tag="pay1")
        l_sb = wp.tile([B, RS * CW], f32, tag="l_sb")
        t0 = l_sb

        gb = small.tile([B, 1], f32, tag="gb")
        nc.gpsimd.dma_start(out=gb[:], in_=dap(consts_d, 0, [[0, B], [1, 1]]))
        msk_use = small.tile([16, 1], f32, tag="msk_use")
        nc.gpsimd.dma_start(out=msk_use[:],
                            in_=dap(consts_d, 1, [[0, 16], [1, 1]]))
        rrW = RS * CW
        NTW = [512, 512, CW - 1024]          # 1250 = 512+512+226
        for s in range(RS):
            for t in range(len(NTW)):
                wdt = NTW[t]
                c0 = s * CW + t * 512
                lt = ps_no.tile([B, 512], f32, tag="nout", name="lt")
                wk = sp.tile([128, 4, 512], f32, tag="wk")
                nc.sync.dma_start(
                    out=wk[:, :, :wdt],
                    in_=dap(rWs_d, c0, [[rrW, 128], [128 * rrW, 4], [1, wdt]]))
                wr = sp.tile([2, 512], f32, tag="wr")
                nc.sync.dma_start(
                    out=wr[:, :wdt],
                    in_=dap(rWs_d, IN_CH * rrW + c0, [[rrW, 2], [1, wdt]]))
                for k in range(4):
                    nc.tensor.matmul(lt[:, :wdt], ryrT[:, k, :],
                                     wk[:, k, :wdt], start=(k == 0), stop=False)
                nc.tensor.matmul(lt[:, :wdt], rrows[:, :B], wr[:, :wdt],
                                 start=False, stop=True)
                nc.scalar.activation(l_sb[:, c0:c0 + wdt], lt[:, :wdt],
                                     AF.Copy, scale=rrstd[:, 0:1])
            top8 = small.tile([B, 8], f32, tag="top8")
            nc.vector.max(top8[:], l_sb[:, s * CW:(s + 1) * CW])
            nc.vector.tensor_copy(pay1[:, s:s + 1], top8[:, 0:1])
            li = small.tile([B, 8], u32, tag="li")
            nc.vector.max_index(li[:], top8[:], l_sb[:, s * CW:(s + 1) * CW])
            lif = small.tile([B, 1], f32, tag="lif")
            nc.vector.tensor_copy(lif[:], li[:, 0:1])
            nc.vector.tensor_scalar_add(pay1[:, 6 + s:7 + s], lif[:],
                                        gb[:, 0:1])
            nm = small.tile([B, 1], f32, tag="nm")
            nc.vector.tensor_scalar_mul(nm[:], top8[:, 0:1], -1.0)
            zc = small.tile([B, 1], f32, tag="zc")
            nc.scalar.activation(t0[:, s * CW:(s + 1) * CW],
                                 l_sb[:, s * CW:(s + 1) * CW], AF.Exp,
                                 bias=nm[:, 0:1], accum_out=zc[:])
            nc.vector.tensor_copy(pay1[:, 3 + s:4 + s], zc[:])
        nc.vector.memset(pay1[:, 9:F1], 0.0)
        nc.sync.dma_start(out=ag1_in[:], in_=pay1[:])
        nc.gpsimd.collective_compute(
            "AllGather", ALU.bypass, replica_groups=[list(range(NC))],
            ins=[ag1_in.opt()], outs=[ag1_out.opt()])

        # ------------------ combine stats ------------------
        comb = work1.tile([B, NC, F1], f32, tag="comb")
        nc.sync.dma_start(
            out=comb[:], in_=dap(ag1_out, 0, [[F1, B], [B * F1, NC], [1, F1]]))
        Mg = small.tile([B, 1], f32, tag="Mg")
        nc.vector.tensor_reduce(Mg[:], comb[:, :, 0:3], axis=AX.XY, op=ALU.max)
        nMg = small.tile([B, 1], f32, tag="nMg")
        nc.vector.tensor_scalar_mul(nMg[:], Mg[:], -1.0)
        et = work1.tile([B, NC, 3], f32, tag="et")
        nc.scalar.activation(et[:], comb[:, :, 0:3], AF.Exp, bias=nMg[:, 0:1])
        nc.vector.tensor_mul(et[:], et[:], comb[:, :, 3:6])
        Zg = small.tile([B, 1], f32, tag="Zg")
        nc.vector.tensor_reduce(Zg[:], et[:], axis=AX.XY, op=ALU.add)
        Zrec = small.tile([B, 1], f32, tag="Zrec")
        nc.vector.reciprocal(Zrec[:], Zg[:])
        combA = comb[:]
        Lmax = small.tile([B, RS], f32, tag="Lmax")
        nc.vector.tensor_reduce(
            Lmax[:],
            bass.AP(tensor=combA.tensor, offset=combA.offset,
                    ap=[combA.ap[0], [1, RS], [F1, NC]]),
            axis=AX.X, op=ALU.max)
        pmax = small.tile([B, RS], f32, tag="pmax")
        nc.scalar.activation(pmax[:], Lmax[:], AF.Exp, bias=nMg[:, 0:1])
        nc.vector.tensor_scalar_mul(pmax[:], pmax[:], Zrec[:, 0:1])
        npmax = small.tile([B, RS], f32, tag="npmax")
        nc.vector.tensor_scalar_mul(npmax[:], pmax[:], -1.0)
        alph = small.tile([B, RS], f32, tag="alph")
        nc.scalar.activation(alph[:], pay1[:, 0:3], AF.Exp, bias=nMg[:, 0:1])
        nc.vector.tensor_scalar_mul(alph[:], alph[:], Zrec[:, 0:1])
        G = small.tile([B, RS], f32, tag="G")
        for s in range(RS):
            mskr = small.tile([B, NC], f32, tag="mskr")
            nc.vector.tensor_scalar(
                out=mskr[:],
                in0=bass.AP(tensor=combA.tensor, offset=combA.offset + s,
                            ap=[combA.ap[0], [F1, NC]]),
                scalar1=Lmax[:, s:s + 1], scalar2=None, op0=ALU.is_equal)
            nc.vector.tensor_mul(
                mskr[:], mskr[:],
                bass.AP(tensor=combA.tensor, offset=combA.offset + 6 + s,
                        ap=[combA.ap[0], [F1, NC]]))
            nc.vector.tensor_reduce(G[:, s:s + 1], mskr[:], axis=AX.X,
                                    op=ALU.add)

        # ------------------ pass2 + AG2 ------------------
        ag2_in = dram.tile([B, F2], f32)
        ag2_out = dram.tile([NC * B, F2], f32)
        pay2 = work1.tile([B, F2], f32, tag="pay2")
        nc.vector.memset(pay2[:], 0.0)
        for s in range(RS):
            q = small.tile([B, 1], f32, tag="q")
            junk = sp.tile([B, CW], f32, tag="wk", name="junk")
            nc.scalar.activation(junk[:], t0[:, s * CW:(s + 1) * CW],
                                 AF.Exp, bias=npmax[:, s:s + 1],
                                 scale=alph[:, s:s + 1], accum_out=q[:])
            nc.vector.tensor_copy(pay2[:, s:s + 1], q[:])
        nc.sync.dma_start(out=ag2_in[:], in_=pay2[:])
        nc.gpsimd.collective_compute(
            "AllGather", ALU.bypass, replica_groups=[list(range(NC))],
            ins=[ag2_in.opt()], outs=[ag2_out.opt()])
        q2 = work1.tile([B, NC, F2], f32, tag="q2")
        nc.sync.dma_start(
            out=q2[:], in_=dap(ag2_out, 0, [[F2, B], [B * F2, NC], [1, F2]]))
        q2b = q2[:]
        qs = small.tile([B, RS], f32, tag="qs")
        nc.vector.tensor_reduce(
            qs[:],
            bass.AP(tensor=q2b.tensor, offset=q2b.offset,
                    ap=[q2b.ap[0], [1, RS], [F2, NC]]),
            axis=AX.X, op=ALU.add)
        rw = small.tile([B, RS], f32, tag="rw")
        nc.vector.reciprocal(rw[:], qs[:])

        # ------------------ gather (uniform; idx select by mask) ----------
        Gsc = dram.tile([B, RS], f32)
        nc.sync.dma_start(out=Gsc[:], in_=G[:])
        g16 = work1.tile([16, 12], f32, tag="g16")
        nc.sync.dma_start(
            out=g16[:], in_=dap(Gsc, 0, [[RS, 16], [1, RS], [16 * RS, 4]]))
        r16 = work1.tile([16, 12], f32, tag="r16")
        nc.sync.dma_start(out=r16[:], in_=ridx16_d[:])
        dif = small.tile([16, 12], f32, tag="dif")
        nc.vector.tensor_sub(dif[:], g16[:], r16[:])
        sel = small.tile([16, 12], f32, tag="sel")
        nc.vector.scalar_tensor_tensor(out=sel[:], in0=dif[:],
                                       scalar=msk_use[:, 0:1], in1=r16[:],
                                       op0=ALU.mult, op1=ALU.add)
        # exact f32 -> i16: add 2^23 so the integer sits in the low mantissa
        # bits, then take the low half of each f32 word.
        sel2 = small.tile([16, 12], f32, tag="sel2")
        nc.vector.tensor_scalar_add(sel2[:], sel[:], 8388608.0)
        selb = sel2[:].bitcast(i16).rearrange("p (c two) -> p c two", two=2)
        idxs = work1.tile([128, 12], i16, tag="idxs")
        nc.vector.memset(idxs[:], 0)
        nc.vector.tensor_copy(idxs[:16, :], selb[:, :, 0])
        # hw requires the 16-partition index block replicated across all
        # 8 gpsimd cores' stripes
        for kk in range(1, 8):
            nc.gpsimd.dma_start(out=idxs[16 * kk:16 * (kk + 1), :],
                                in_=idxs[0:16, :])
        gath = sp.tile([128, 2, SLOT], f32, tag="wk", name="gath")
        nc.gpsimd.dma_gather(gath[:], memory_d[:], idxs[:],
                             RS * B, RS * B, SLOT)
        gathb = work1.tile([128, 2, SLOT], bf16, tag="gathb")
        nc.vector.tensor_copy(gathb[:].rearrange("p a c -> p (a c)"),
                              gath[:].rearrange("p a c -> p (a c)"))

        # ------------------ update heads (uniform) ------------------
        wU = load_head("bigA", bf16, bigd["W1a"], bigd["W2a"], bigd["Wfa"],
                       bigd["Wna"], 2 * SLOT, SLOT)
        wS = load_head("sm", bf16, sW1_d, sW2_d, sWf_d, sWn_d, IN_CH, RS)
        uw_sb = small.tile([B, RS], f32, tag="uw_sb")
        big_head(wS, [(xTb[:], 4)], B, None, AF.Sigmoid, uw_sb[:])

        pay3a = work1.tile([128, 4, B], bf16, tag="pay3a")
        pay3b = work1.tile([128, 8, B], bf16, tag="pay3b")

        def mix_and_pack(dst, u_ap, mem_bm, uwcol, koff):
            bp = u_ap.base_partition()
            M = u_ap.shape[0]
            d_t = work1.tile([128, SLOT], f32, tag="dmix")
            d = d_t[bp:bp + M, :]
            nc.vector.tensor_sub(d, u_ap, mem_bm)
            r_t = work1.tile([128, SLOT], f32, tag="rmix")
            r = r_t[bp:bp + M, :]
            nc.vector.scalar_tensor_tensor(out=r, in0=d, scalar=uwcol,
                                           in1=mem_bm, op0=ALU.mult,
                                           op1=ALU.add)
            rb_t = work1.tile([128, SLOT], bf16, tag="rbmix")
            rb = rb_t[bp:bp + M, :]
            nc.vector.tensor_copy(rb, r)
            transpose_in(dst, rb, bf16, koff=koff)

        # u_0 -> pay3a
        m0T = work1.tile([128, 4, B], bf16, tag="m0T")
        transpose_in(m0T, gathb[0:B, 0, :], bf16)
        u0 = work1.tile([B, SLOT], f32, tag="u0")
        big_head(wU, [(m0T[:], 4), (xTb[:], 4)], B, None, AF.Relu, u0[:])
        mix_and_pack(pay3a, u0[:], gathb[0:B, 0, :], uw_sb[:, 0:1], 0)

        ag3a_in = dram.tile([128, 4 * B], bf16)
        ag3a_out = dram.tile([NC * 128, 4 * B], bf16)
        nc.sync.dma_start(out=ag3a_in[:],
                          in_=pay3a[:].rearrange("p a b -> p (a b)"))
        nc.gpsimd.collective_compute(
            "AllGather", ALU.bypass, replica_groups=[list(range(NC))],
            ins=[ag3a_in.opt()], outs=[ag3a_out.opt()])

        # u_1, u_2 (M=128) -> pay3b; col 0:64 = s2 (base 0), 64:128 = s1
        m12T = work1.tile([128, 4, 2 * B], bf16, tag="m12T")
        transpose_in(m12T, gathb[0:B, 1, :], bf16, coff=0)
        transpose_in(m12T, gathb[B:2 * B, 0, :], bf16, coff=B)
        u12 = work1.tile([2 * B, SLOT], f32, tag="u12")
        big_head(wU, [(m12T[:], 4), (xTb[:], 4)], 2 * B, None, AF.Relu, u12[:])
        mix_and_pack(pay3b, u12[0:B, :], gathb[0:B, 1, :],
                     uw_sb[:, 2:3], 4)
        uwsh = work1.tile([128, 1], f32, tag="uwsh")
        nc.sync.dma_start(out=uwsh[64:128, :], in_=uw_sb[:, 1:2])
        mix_and_pack(pay3b, u12[B:2 * B, :], gathb[B:2 * B, 0, :],
                     uwsh[64:128, 0:1], 0)

        ag3b_in = dram.tile([128, 8 * B], bf16)
        ag3b_out = dram.tile([NC * 128, 8 * B], bf16)
        nc.sync.dma_start(out=ag3b_in[:],
                          in_=pay3b[:].rearrange("p a b -> p (a b)"))
        nc.gpsimd.collective_compute(
            "AllGather", ALU.bypass, replica_groups=[list(range(NC))],
            ins=[ag3b_in.opt()], outs=[ag3b_out.opt()])

        # ------------------ chain (uniform; real only on core 7) ----------
        wB_h = load_head("bigB", bf16, bigd["W1b"], bigd["W2b"], bigd["Wfb"],
                         bigd["Wnb"], 2 * SLOT, SLOT)
        rT = work1.tile([128, 12, B], bf16, tag="rT")
        nc.sync.dma_start(
            out=rT[:, 0:4, :].rearrange("p a b -> p (a b)"),
            in_=ag3a_out[U_CORE * 128:(U_CORE + 1) * 128, :])
        nc.sync.dma_start(
            out=rT[:, 4:12, :].rearrange("p a b -> p (a b)"),
            in_=ag3b_out[U_CORE * 128:(U_CORE + 1) * 128, :])
        r2T = work1.tile([128, 12, B], bf16, tag="r2T")
        nc.sync.dma_start(
            out=r2T[:, 0:4, :].rearrange("p a b -> p (a b)"),
            in_=ag3a_out[U2_CORE * 128:(U2_CORE + 1) * 128, :])
        nc.sync.dma_start(
            out=r2T[:, 4:12, :].rearrange("p a b -> p (a b)"),
            in_=ag3b_out[U2_CORE * 128:(U2_CORE + 1) * 128, :])
        m_sb = wp.tile([B, SLOT], f32, tag="x_msb", name="m_sb")
        nc.vector.memset(m_sb[:], 0.0)
        mT = wp.tile([128, 4, B], bf16, tag="mT")
        nc.vector.memset(mT[:].rearrange("p a b -> p (a b)"), 0.0)
        for step in range(6):
            wH = wU if step % 2 == 0 else wB_h
            s = step // 2
            src = rT if step % 2 == 0 else r2T
            seg = [(src[:, s * 4:(s + 1) * 4, :], 4), (mT[:], 4)]
            t_sb = work1.tile([B, SLOT], f32, tag="t_sb")
            big_head(wH, seg, B, rw[:, s:s + 1], AF.Relu, t_sb[:])
            nc.vector.tensor_add(m_sb[:], m_sb[:], t_sb[:])
            if step < 5:
                mb = work1.tile([B, SLOT], bf16, tag="rbmix", name="mb")
                nc.vector.tensor_copy(mb[:], m_sb[:])
                transpose_in(mT, mb[:], bf16)
        nc.sync.dma_start(out=out_d[:], in_=m_sb[:])

        dbg = work1.tile([B, 16], f32, tag="dbg")
        nc.vector.memset(dbg[:], 0.0)
        nc.vector.tensor_copy(dbg[:, 0:3], G[:])
        nc.vector.tensor_copy(dbg[:, 3:6], rw[:])
        nc.vector.tensor_copy(dbg[:, 6:9], pmax[:])
        nc.vector.tensor_copy(dbg[:, 9:10], Mg[:])
        nc.vector.tensor_copy(dbg[:, 10:11], Zg[:])
        nc.sync.dma_start(out=dbg_d[:], in_=dbg[:])

    return nc


def kernel(inputs, memory, random_indices, params):
    from concourse.bass_utils import run_bass_kernel_spmd
    in_maps = _build_inputs(inputs, memory, random_indices, params)
    nc = _build_nc()
    if not nc.is_finalized():
        nc.finalize()
    res = run_bass_kernel_spmd(nc, in_maps, core_ids=list(range(NC)),
                               trace=bool(int(os.environ.get("KTRACE", "0"))))
    kernel.last_results = res
    return res.results[CH_CORE]["out"].copy()


def kernel_timed(inputs, memory, random_indices, params, iters=3):
    """Run once for outputs, then time warm device executions (device-
    resident inputs, donated outputs). Returns (out, best_exec_seconds)."""
    import time
    import jax
    import numpy as np
    from jax.sharding import Mesh, PartitionSpec
    from jax.experimental.shard_map import shard_map
    import concourse.mybir as mybir
    from concourse.bass2jax import (_bass_exec_p, partition_id_tensor,
                                    install_neuronx_cc_hook)

    in_maps = _build_inputs(inputs, memory, random_indices, params)
    nc = _build_nc()
    if not nc.is_finalized():
        nc.finalize()
    install_neuronx_cc_hook()
    partition_name = nc.partition_id_tensor.name if nc.partition_id_tensor         else None
    in_names, out_names, out_avals, zero_outs = [], [], [], []
    for alloc in nc.m.functions[0].allocations:
        if not isinstance(alloc, mybir.MemoryLocationSet):
            continue
        name = alloc.memorylocations[0].name
        if alloc.kind == "ExternalInput":
            if name != partition_name:
                in_names.append(name)
        elif alloc.kind == "ExternalOutput":
            shape = tuple(alloc.tensor_shape)
            dtype = mybir.dt.np(alloc.dtype)
            out_names.append(name)
            out_avals.append(jax.core.ShapedArray(shape, dtype))
            zero_outs.append(np.zeros(shape, dtype))
    n_params = len(in_names)
    n_outs = len(out_avals)
    all_names = list(in_names) + list(out_names)
    if partition_name is not None:
        all_names.append(partition_name)
    donate = tuple(range(n_params, n_params + n_outs))

    def _body(*args):
        operands = list(args)
        if partition_name is not None:
            operands.append(partition_id_tensor())
        return tuple(_bass_exec_p.bind(
            *operands, out_avals=tuple(out_avals), in_names=tuple(all_names),
            out_names=tuple(out_names), lowering_input_output_aliases=(),
            sim_require_finite=True, sim_require_nnan=True, nc=nc))

    devices = jax.devices()[:NC]
    mesh = Mesh(np.asarray(devices), ("core",))
    sharded = jax.jit(
        shard_map(_body, mesh=mesh,
                  in_specs=(PartitionSpec("core"),) * (n_params + n_outs),
                  out_specs=(PartitionSpec("core"),) * n_outs,
                  check_rep=False),
        donate_argnums=donate, keep_unused=True)
    concat_in = [np.concatenate([np.asarray(in_maps[c][in_names[i]])
                                 for c in range(NC)], axis=0)
                 for i in range(n_params)]
    sh = jax.sharding.NamedSharding(mesh, PartitionSpec("core"))
    dev_in = [jax.device_put(a, sh) for a in concat_in]

    def zeros():
        return [jax.device_put(
            np.zeros((NC * z.shape[0], *z.shape[1:]), z.dtype), sh)
            for z in zero_outs]

    out_arrs = jax.block_until_ready(sharded(*dev_in, *zeros()))
    out = np.asarray(out_arrs[out_names.index("out")]).reshape(
        NC, B, SLOT)[CH_CORE].copy()
    best = None
    try:
        for _ in range(iters):
            zo = zeros()
            jax.block_until_ready(zo)
            t0 = time.perf_counter()
            r = sharded(*dev_in, *zo)
            jax.block_until_ready(r)
            dt = time.perf_counter() - t0
            best = dt if best is None else min(best, dt)
    except Exception as e:
        print("timing reruns failed (%s); reporting first-run estimate" % e)
        best = best or float("nan")
    return out, best
